# revision 57
# baseline (speedup 1.0000x reference)
"""Causal attention kernel for TRN2, 8 NeuronCores (SPMD).

Problem:  x[4096,2048] f32; q = x@Wq.T, k = x@Wk.T (d_head=128),
          scores = q@k.T causal-masked, attn = softmax(scores),
          out = (attn @ x) @ W2.T.

Sharding: sequence-parallel over queries with stride-8 interleave:
  core c owns queries {8m+c : m=0..511}.  For key tile kt (128 keys),
  every core has exactly 512-16*kt valid queries -- a contiguous tail
  slice of its query columns -- so the SPMD program is identical on all
  cores (no dynamic control flow, no collectives) and causal work is
  perfectly balanced.

Precision: fp16 inputs for the q/k projections and the score matmul
  (fp32 PSUM accumulation), unnormalized softmax (exp without
  max-subtraction: scores are bounded ~|s|<70 for unit-normal inputs,
  safely inside fp32 exp range), attention weights in bf16 (bf16 has
  fp32 exponent range, needed for exp(s) up to ~1e28), V and W2
  matmuls in bf16/fp16, normalization by the softmax row-sum applied
  at the attn_out eviction (keeps fp16 in range).

Scheduling (v3, ~212-217us on HW; measured facts from perfetto/NTFF):
  * PE busy ~170us of ~215: the kernel is tensor-bound.  Per-matmul
    LDWEIGHTS is ~98ns and hides under the moving phase only when the
    moving width exceeds ~235 cycles; the kT projection (512 matmuls
    of 128-wide moving) is LD-bound at ~1.57us/key-tile.
  * PE p-state: after any idle gap the engine runs at ~2x cycle time
    for up to 3us, so scheduling stalls inflate busy time, not just
    idle time.  Keeping streams strictly ordered (xtp -> xv -> w2r,
    all on the sync queue) measured faster than interleaving them:
    interleaved issues serialize transfers and starve the fused phase
    (232us vs 212us, measured both ways).
  * xtp is host-packed into 2MB chunk-major tiles (4 key groups per
    chunk, double-buffered pool) so the stream needs 8 ring-gated
    issues instead of 32; xq/wq bytes lead the stream so the qT
    projection (first in the Tile-scheduled PE order) isn't starved.
  * The Tile static scheduler largely follows emission order on the
    PE queue -- emit in intended execution order (kT[0] first, qT
    woven into kg 0/1, scores trailing by two tiles, V waves after
    the fused loop so the in-order PE queue can't wedge on xv).
  * V-wave / W2 choreography from v1 (pso1+g2 8-bank wave kt<16,
    bc oc 8-15 full range, g4 merge, W2 with ic order 8..15 first).
  * Rejected (measured or analyzed): AllGather-sharded kT (~100us
    for 0.5MB), fp8 anywhere on the main datapath (3.6% rms rel err
    vs the 2e-2 gate), on-chip PE transpose of x to kill the second
    x load (adds ~50us of LD-bound PE), per-queue DMA streams beyond
    sync (head-of-line or bus contention), V stationary/moving flip
    (breaks the [d, q] layout W2 consumes).
"""

from contextlib import ExitStack

import numpy as np
import ml_dtypes

import concourse.bass as bass
import concourse.bacc as bacc
import concourse.mybir as mybir
import concourse.tile as tile
from concourse.bass_utils import run_bass_kernel_spmd
from concourse.tile_rust import add_dep_helper

N_CTX = 4096
D_MODEL = 2048
D_HEAD = 128
NCORES = 8
QPC = N_CTX // NCORES          # 512 queries per core
NKT = N_CTX // 128             # 32 key tiles
NDM = D_MODEL // 128           # 16 d_model chunks
KG = 128                       # kT projection key-group width
NKG = N_CTX // KG
MASK_NEG = -1.0e30

F16 = mybir.dt.float16
BF16 = mybir.dt.bfloat16
F32 = mybir.dt.float32


def _widths():
    # valid query-column width per key tile (tail slice [512-w : 512] of qT)
    return [QPC - 16 * kt for kt in range(NKT)]


def build_program():
    nc = bacc.Bacc(trn_type="TRN2", target_bir_lowering=False, debug=False)

    # ---- DRAM parameters (identical shapes on all cores; data differs) ----
    # xqr[r, 512*ic + m] = x[8m+c, 128*ic + r]   (own-query columns, packed)
    xqr = nc.declare_dram_parameter("xqr", [128, NDM * QPC], F16, isOutput=False)
    # xtp[ch][r, 2048*j + KG*ic + n] = x[KG*(4*ch+j) + n, 128*ic + r]
    # (chunk-major: 4 key-groups per contiguous 2MB chunk)
    xtp = nc.declare_dram_parameter(
        "xtp", [NKG // 4, 128, 4 * NDM * KG], F16, isOutput=False
    )
    # xv = x (natural layout), bf16
    xv = nc.declare_dram_parameter("xv", [N_CTX, D_MODEL], BF16, isOutput=False)
    # wqr[r, 128*ic + h] = Wq[h, 128*ic + r]; same for wkr
    wqr = nc.declare_dram_parameter("wqr", [128, D_MODEL], F16, isOutput=False)
    wkr = nc.declare_dram_parameter("wkr", [128, D_MODEL], F16, isOutput=False)
    # w2r[oc][r, 128*ic + o] = W2[128*oc + o, 128*ic + r]
    w2r = nc.declare_dram_parameter("w2r", [NDM, 128, D_MODEL], F16, isOutput=False)
    maskb = nc.declare_dram_parameter("maskb", [128, 16], F32, isOutput=False)
    outT = nc.declare_dram_parameter("outT", [D_MODEL, QPC], F16, isOutput=True)

    W = _widths()
    HK = NKT // 2

    with tile.TileContext(nc) as tc:
        with (
            tc.tile_pool(name="static", bufs=1) as st,
            tc.tile_pool(name="xvp1", bufs=24) as xvp1,
            tc.tile_pool(name="xvp2", bufs=8) as xvp2,
            tc.tile_pool(name="ktpool", bufs=8) as ktp,
            tc.tile_pool(name="atpool", bufs=1) as atp,
        ):
            qT_sb = st.tile([128, QPC], F16, tag="qT")
            ones_sb = st.tile([128, 1], BF16, tag="ones")
            mask_sb = st.tile([128, 16], F32, tag="mask")
            recip_sb = st.tile([128, QPC], F32, tag="recip")
            nc.vector.memset(ones_sb[:], 1.0)

            # ---- critical small loads first (sync queue order == byte order)
            nc.sync.dma_start(out=mask_sb[:], in_=maskb[:])

            es1 = ExitStack()  # xts staging + wk/wq: freed before p34
            xts = es1.enter_context(tc.tile_pool(name="xts", bufs=2))
            wkp = es1.enter_context(tc.tile_pool(name="wkp", bufs=1))
            wk_sb = wkp.tile([128, D_MODEL], F16, tag="wk")
            wq_sb = wkp.tile([128, D_MODEL], F16, tag="wq")
            es_xq = ExitStack()  # one-shot xq staging, closed right after qT
            xqp = es_xq.enter_context(tc.tile_pool(name="xqp", bufs=1))
            xq_sb = xqp.tile([128, NDM * QPC], F16, tag="xq")

            # byte order: xq h1 -> wkr -> wqr -> xtp kg 0..3 (per-kg
            # sub-DMAs of chunk 0) -> xq h2 -> xtp chunks 1..7 (2MB each;
            # 5.0us transfer + 0.6us issue = ~1.4us/tile delivery, faster
            # than the ~1.67us/tile PE consumption, so the stream runs
            # ahead and the fused phase never stalls) -> xv singles.
            # xts is one big 8-slot tile; chunk c lands in slots
            # (4c)%8..(4c)%8+3, so every chunk is slot-contiguous.
            # head bytes finely interleaved: qT and kT inputs land
            # progressively from ~10us instead of one 12us xq stall
            QQ = NDM * QPC // 4
            HQ = NDM * QPC // 2
            nc.sync.dma_start(out=wk_sb[:], in_=wkr[:])
            nc.sync.dma_start(out=wq_sb[:], in_=wqr[:])
            nc.sync.dma_start(out=xq_sb[:, 0:QQ], in_=xqr[:, 0:QQ])

            # chunk tiles: 4 key groups each, double-buffered pool so Tile's
            # slot versioning orders chunk c's DMA after chunk c-2's reads
            SW = NDM * KG  # slot width (columns per key group)
            ch_t = []
            ch_dma = []

            def xts_slot(kg):
                return ch_t[kg // 4][:, (kg % 4) * SW : (kg % 4 + 1) * SW]

            t0 = xts.tile([128, 4 * SW], F16, tag="xts", name="xts_ch0")
            ch_t.append(t0)
            d = nc.sync.dma_start(out=t0[:, 0:SW], in_=xtp[0][:, 0:SW])
            nc.sync.dma_start(out=xq_sb[:, QQ:HQ], in_=xqr[:, QQ:HQ])
            d = nc.sync.dma_start(out=t0[:, SW : 2 * SW], in_=xtp[0][:, SW : 2 * SW])
            nc.sync.dma_start(out=xq_sb[:, HQ:], in_=xqr[:, HQ:])
            for kg in range(2, 4):
                d = nc.sync.dma_start(
                    out=t0[:, kg * SW : (kg + 1) * SW],
                    in_=xtp[0][:, kg * SW : (kg + 1) * SW],
                )
            ch_dma.append(d)
            for ch in range(1, NKG // 4):
                t = xts.tile([128, 4 * SW], F16, tag="xts", name=f"xts_ch{ch}")
                ch_t.append(t)
                nc.sync.dma_start(
                    out=t[:, 0 : 2 * SW], in_=xtp[ch][:, 0 : 2 * SW]
                )
                ch_dma.append(
                    nc.sync.dma_start(
                        out=t[:, 2 * SW : 4 * SW], in_=xtp[ch][:, 2 * SW : 4 * SW]
                    )
                )

            xv_t = []
            for kt in range(NKT):
                pool = xvp1 if kt < 24 else xvp2
                t = pool.tile([128, D_MODEL], BF16, tag="xv", name=f"xv{kt}")
                nc.sync.dma_start(out=t[:], in_=xv[128 * kt : 128 * (kt + 1), :])
                xv_t.append(t)

            # ---- fused pipeline: kT proj / qT proj / scores / exp / denom.
            # Emission order == PE execution order: kT[0] first (its inputs
            # land earliest), qT halves woven into kg 0/1, scores trailing
            # the kT projection by two tiles. ----
            at_t = []
            with tc.tile_pool(name="psk", bufs=3, space="PSUM") as pskp:
                es_psq = ExitStack()  # psq bank, freed for pss/psd after qT
                psqp = es_psq.enter_context(
                    tc.tile_pool(name="psq", bufs=1, space="PSUM")
                )
                psq = psqp.tile([128, QPC], F32, tag="psq")
                es_sd = ExitStack()  # pss + psd, opened once psq closes
                pssp = None
                psd = None

                def emit_scores(kt):
                    w = W[kt]
                    ps = pssp.tile([128, 512], F32, tag="pss", name=f"pss{kt}")
                    nc.tensor.matmul(
                        ps[:, :w],
                        kt_tiles[kt][:],
                        qT_sb[:, QPC - w : QPC],
                        start=True, stop=True,
                    )
                    nc.vector.tensor_add(ps[:, :16], ps[:, :16], mask_sb[:])
                    at = atp.tile([128, w], BF16, tag=f"at{kt}")
                    nc.scalar.activation(
                        at[:], ps[:, :w], mybir.ActivationFunctionType.Exp
                    )
                    at_t.append(at)
                    nc.tensor.matmul(
                        psd[0:1, QPC - w : QPC],
                        ones_sb[:],
                        at[:],
                        start=(kt == 0), stop=(kt == NKT - 1),
                    )

                kt_tiles = []
                for kg in range(NKG):
                    psk = pskp.tile([128, KG], F32, tag="psk", name=f"psk{kg}")
                    slot = xts_slot(kg)
                    for ic in range(NDM):
                        nc.tensor.matmul(
                            psk[:],
                            wk_sb[:, 128 * ic : 128 * (ic + 1)],
                            slot[:, KG * ic : KG * (ic + 1)],
                            start=(ic == 0), stop=(ic == NDM - 1),
                        )
                    ktile = ktp.tile([128, KG], F16, tag="kt", name=f"kt{kg}")
                    nc.vector.tensor_copy(ktile[:], psk[:])
                    kt_tiles.append(ktile)

                    # weave the qT projection into the first two kg steps
                    if kg == 0:
                        for ic in range(NDM // 2):
                            nc.tensor.matmul(
                                psq[:],
                                wq_sb[:, 128 * ic : 128 * (ic + 1)],
                                xq_sb[:, QPC * ic : QPC * (ic + 1)],
                                start=(ic == 0), stop=False,
                            )
                    elif kg == 1:
                        for ic in range(NDM // 2, NDM):
                            nc.tensor.matmul(
                                psq[:],
                                wq_sb[:, 128 * ic : 128 * (ic + 1)],
                                xq_sb[:, QPC * ic : QPC * (ic + 1)],
                                start=False, stop=(ic == NDM - 1),
                            )
                        nc.vector.tensor_copy(qT_sb[:], psq[:])
                        es_xq.close()  # free the xq staging (SBUF)
                        es_psq.close()  # free the psq bank ...
                        pssp = es_sd.enter_context(
                            tc.tile_pool(name="pss", bufs=2, space="PSUM")
                        )
                        psdp = es_sd.enter_context(
                            tc.tile_pool(name="psd", bufs=1, space="PSUM")
                        )
                        psd = psdp.tile([1, QPC], F32, tag="psd")
                    if kg >= 2:
                        emit_scores(kg - 2)
                emit_scores(NKT - 2)
                emit_scores(NKT - 1)

                # softmax denominators -> reciprocal (reads psd before the
                # pool closes; broadcast follows outside)
                nc.vector.reciprocal(recip_sb[0:1, :], psd[0:1, :])
                es_sd.close()

            es1.close()  # free wk/wq + xts SBUF for the aoT / W2 pools
            nc.gpsimd.partition_broadcast(recip_sb[:], recip_sb[0:1, :])

            with tc.tile_pool(name="p34", bufs=1) as p34:
                ao_t = {}

                # ---- oc 0-7, first key half (kt 0-15): pso1 + g2 run as
                # one 8-bank wave on the PSUM freed by the fused pools ----
                es2 = ExitStack()  # psv1: freed between the pso1 and g2 evictions
                psv1 = es2.enter_context(
                    tc.tile_pool(name="psv1", bufs=4, space="PSUM", side="right")
                )
                pso1 = [
                    psv1.tile([128, QPC], F32, tag="pso1", name=f"pso1_{j}")
                    for j in range(4)
                ]
                with tc.tile_pool(name="g2", bufs=4, space="PSUM") as g2p:
                    g2 = {
                        oc: g2p.tile([128, QPC], F32, tag="g2", name=f"g2_{oc}")
                        for oc in range(4, 8)
                    }
                    for kt in range(HK):
                        w = W[kt]
                        for j in range(4):
                            nc.tensor.matmul(
                                pso1[j][:, QPC - w : QPC],
                                xv_t[kt][:, 128 * j : 128 * (j + 1)],
                                at_t[kt][:],
                                start=(kt == 0), stop=(kt == HK - 1),
                            )
                        for oc in range(4, 8):
                            nc.tensor.matmul(
                                g2[oc][:, QPC - w : QPC],
                                xv_t[kt][:, 128 * oc : 128 * (oc + 1)],
                                at_t[kt][:],
                                start=(kt == 0), stop=(kt == HK - 1),
                            )
                    # early normalized evictions for oc 0-7 (partial over the
                    # first key half; exact for queries m<256 by causality)
                    for j in range(4):
                        t = p34.tile([128, QPC], F16, tag=f"ao{j}")
                        nc.vector.tensor_mul(t[:], pso1[j][:], recip_sb[:])
                        ao_t[j] = t
                    es2.close()  # release the fused V banks
                    for oc in range(4, 8):
                        t = p34.tile([128, QPC], F16, tag=f"ao{oc}")
                        nc.vector.tensor_mul(t[:], g2[oc][:], recip_sb[:])
                        ao_t[oc] = t

                # ---- oc 8-15, FULL key range: one combined 8-bank sweep;
                # bcR's evictions run first so g4 can open on the right
                # banks while bcL's evictions still drain on DVE ----
                es_bcL = ExitStack()
                bcL = es_bcL.enter_context(
                    tc.tile_pool(name="bcL", bufs=4, space="PSUM")
                )
                es_bcR = ExitStack()
                bcR = es_bcR.enter_context(
                    tc.tile_pool(name="bcR", bufs=4, space="PSUM", side="right")
                )
                bc = {}
                for i, oc in enumerate(range(8, NDM)):
                    pool = bcL if i < 4 else bcR
                    bc[oc] = pool.tile(
                        [128, QPC], F32, tag="bc", name=f"bc_{oc}"
                    )
                for kt in range(NKT):
                    w = W[kt]
                    for oc in range(8, NDM):
                        nc.tensor.matmul(
                            bc[oc][:, QPC - w : QPC],
                            xv_t[kt][:, 128 * oc : 128 * (oc + 1)],
                            at_t[kt][:],
                            start=(kt == 0), stop=(kt == NKT - 1),
                        )
                for oc in range(12, NDM):
                    t = p34.tile([128, QPC], F16, tag=f"ao{oc}")
                    nc.vector.tensor_mul(t[:], bc[oc][:], recip_sb[:])
                    ao_t[oc] = t
                es_bcR.close()
                for oc in range(8, 12):
                    t = p34.tile([128, QPC], F16, tag=f"ao{oc}")
                    nc.vector.tensor_mul(t[:], bc[oc][:], recip_sb[:])
                    ao_t[oc] = t
                es_bcL.close()

                # ---- oc 0-7, second key half (kt 16-31, queries [256:512]
                # only): dense from resident xv, merged into the early aos ----
                with (
                    tc.tile_pool(name="g4", bufs=4, space="PSUM", side="right") as g4p,
                    tc.tile_pool(name="tmr", bufs=4) as tmr,
                ):
                    for ocs in (range(0, 4), range(4, 8)):
                        g4 = {
                            oc: g4p.tile(
                                [128, QPC // 2], F32, tag="g4", name=f"g4_{oc}"
                            )
                            for oc in ocs
                        }
                        for kt in range(HK, NKT):
                            w = W[kt]
                            for oc in ocs:
                                nc.tensor.matmul(
                                    g4[oc][:, QPC // 2 - w : QPC // 2],
                                    xv_t[kt][:, 128 * oc : 128 * (oc + 1)],
                                    at_t[kt][:],
                                    start=(kt == HK), stop=(kt == NKT - 1),
                                )
                        for oc in ocs:
                            tm = tmr.tile([128, QPC // 2], F16, tag="tm")
                            nc.vector.tensor_mul(
                                tm[:], g4[oc][:], recip_sb[:, QPC // 2 :]
                            )
                            nc.vector.tensor_add(
                                ao_t[oc][:, QPC // 2 :],
                                ao_t[oc][:, QPC // 2 :],
                                tm[:],
                            )

                    # ---- W2: outT = W2T.T @ attn_outT.  ic order 8..15 first
                    # (those aos finish earliest), 0..7 after the merges ----
                    with (
                        tc.tile_pool(name="w2s", bufs=4) as w2s,
                        tc.tile_pool(name="outs", bufs=4) as outs,
                        tc.tile_pool(name="ps4", bufs=4, space="PSUM") as ps4,
                    ):
                        ic_order = list(range(8, NDM)) + list(range(0, 8))
                        for oc in range(NDM):
                            tw = w2s.tile([128, D_MODEL], F16, tag="w2")
                            nc.sync.dma_start(out=tw[:], in_=w2r[oc])
                            ps = ps4.tile([128, QPC], F32, tag="ps4")
                            for i, ic in enumerate(ic_order):
                                nc.tensor.matmul(
                                    ps[:],
                                    tw[:, 128 * ic : 128 * (ic + 1)],
                                    ao_t[ic][:],
                                    start=(i == 0), stop=(i == NDM - 1),
                                )
                            t = outs.tile([128, QPC], F16, tag="out")
                            nc.vector.tensor_copy(t[:], ps[:])
                            nc.scalar.dma_start(
                                out=outT[128 * oc : 128 * (oc + 1), :], in_=t[:]
                            )

    nc.compile()
    return nc


def prepare_inputs(x, Wk, Wq, W2):
    """Host-side sharding/layout prep. Returns in_maps for the 8 cores."""
    x = np.asarray(x, dtype=np.float32)
    Wk = np.asarray(Wk, dtype=np.float32)
    Wq = np.asarray(Wq, dtype=np.float32)
    W2 = np.asarray(W2, dtype=np.float32)

    xT16 = np.ascontiguousarray(x.T).astype(np.float16)          # [D, N]
    # per-kg tiles: t[kg, r, KG*ic + n] = xT[128*ic + r, KG*kg + n],
    # then packed chunk-major: xtp[ch, r, SW*j + col] = t[4*ch + j, r, col]
    t = xT16.reshape(NDM, 128, NKG, KG).transpose(2, 1, 0, 3).reshape(NKG, 128, NDM * KG)
    xtp = np.ascontiguousarray(
        t.reshape(NKG // 4, 4, 128, NDM * KG).transpose(0, 2, 1, 3)
        .reshape(NKG // 4, 128, 4 * NDM * KG)
    )
    xv16 = x.astype(ml_dtypes.bfloat16)                          # [N, D]

    def pack_chunks(aT, width):
        # aT [D_MODEL, width] -> [128, NDM*width]: out[r, width*ic + c] = aT[128ic+r, c]
        return np.ascontiguousarray(
            aT.reshape(NDM, 128, width).transpose(1, 0, 2).reshape(128, NDM * width)
        )

    wqr = pack_chunks(np.ascontiguousarray(Wq.T).astype(np.float16), D_HEAD)
    wkr = pack_chunks(np.ascontiguousarray(Wk.T).astype(np.float16), D_HEAD)
    # w2r[oc, r, 128*ic + o] = W2T[128ic+r, 128oc+o]
    w2T = np.ascontiguousarray(W2.T).astype(np.float16)
    w2r = np.ascontiguousarray(
        w2T.reshape(NDM, 128, NDM, 128).transpose(2, 1, 0, 3).reshape(NDM, 128, D_MODEL)
    )

    in_maps = []
    for c in range(NCORES):
        xqT = np.ascontiguousarray(x[c::NCORES].T).astype(np.float16)  # [D, QPC]
        xqr_c = pack_chunks(xqT, QPC)
        mask = np.zeros((128, 16), dtype=np.float32)
        j = np.arange(128)[:, None]
        t = np.arange(16)[None, :]
        mask[j > 8 * t + c] = MASK_NEG
        in_maps.append(
            {
                "xqr": xqr_c,
                "xtp": xtp,
                "xv": xv16,
                "wqr": wqr,
                "wkr": wkr,
                "w2r": w2r,
                "maskb": mask,
            }
        )
    return in_maps


def assemble_output(results):
    res = np.stack([np.asarray(results[c]["outT"]).astype(np.float32) for c in range(NCORES)])
    # [c, d, m] -> out[8m+c, d]
    return np.ascontiguousarray(res.transpose(2, 0, 1).reshape(N_CTX, D_MODEL))


_CACHED = {}


def kernel(x, Wk, Wq, W2, _trace=False):
    if "nc" not in _CACHED:
        _CACHED["nc"] = build_program()
    nc = _CACHED["nc"]
    in_maps = prepare_inputs(x, Wk, Wq, W2)
    res = run_bass_kernel_spmd(nc, in_maps, core_ids=list(range(NCORES)), trace=_trace)
    out = assemble_output(res.results)
    if _trace:
        return out, res
    return out


# revision 59
# speedup vs baseline: 1.2000x; 1.2000x over previous
"""Causal attention kernel for TRN2, 8 NeuronCores (SPMD).

Problem:  x[4096,2048] f32; q = x@Wq.T, k = x@Wk.T (d_head=128),
          scores = q@k.T causal-masked, attn = softmax(scores),
          out = (attn @ x) @ W2.T.

Sharding: sequence-parallel over queries with stride-8 interleave:
  core c owns queries {8m+c : m=0..511}.  For key tile kt (128 keys),
  every core has exactly 512-16*kt valid queries -- a contiguous tail
  slice of its query columns -- so the SPMD program is identical on all
  cores (no dynamic control flow, no collectives) and causal work is
  perfectly balanced.

Precision: fp16 inputs for the q/k projections and the score matmul
  (fp32 PSUM accumulation), unnormalized softmax (exp without
  max-subtraction: scores are bounded ~|s|<70 for unit-normal inputs,
  safely inside fp32 exp range), attention weights in bf16 (bf16 has
  fp32 exponent range, needed for exp(s) up to ~1e28), V and W2
  matmuls in bf16/fp16, normalization by the softmax row-sum applied
  at the attn_out eviction (keeps fp16 in range).

Scheduling (v4: 209.2us vs 217.5us same-window A/B; 206.5us was the
  best clean-device reading for the previous build.  The device
  throttle-oscillates 206-247us for identical code, so only
  back-to-back cached-NEFF comparisons are meaningful.  On top of the
  facts below, v4 adds: psk bufs=3 + pss bufs=2 with psv1 opened
  post-fused (kills PSUM slot-turnaround stalls, ~8-12us), a finely
  interleaved head byte order (wkr/wqr, then xq quarters woven into
  chunk-0's per-kg sub-DMAs), and chunk-half DMAs (2x1MB per 2MB
  chunk) so each chunk's first key groups land ~2.5us earlier
  (together ~8us in controlled A/B).

Measured facts from perfetto/NTFF (v3 baseline analysis):
  * PE busy ~170us of ~215: the kernel is tensor-bound.  Per-matmul
    LDWEIGHTS is ~98ns and hides under the moving phase only when the
    moving width exceeds ~235 cycles; the kT projection (512 matmuls
    of 128-wide moving) is LD-bound at ~1.57us/key-tile.
  * PE p-state: after any idle gap the engine runs at ~2x cycle time
    for up to 3us, so scheduling stalls inflate busy time, not just
    idle time.  Keeping streams strictly ordered (xtp -> xv -> w2r,
    all on the sync queue) measured faster than interleaving them:
    interleaved issues serialize transfers and starve the fused phase
    (232us vs 212us, measured both ways).
  * xtp is host-packed into 2MB chunk-major tiles (4 key groups per
    chunk, double-buffered pool) so the stream needs 8 ring-gated
    issues instead of 32; xq/wq bytes lead the stream so the qT
    projection (first in the Tile-scheduled PE order) isn't starved.
  * The Tile static scheduler largely follows emission order on the
    PE queue -- emit in intended execution order (kT[0] first, qT
    woven into kg 0/1, scores trailing by two tiles, V waves after
    the fused loop so the in-order PE queue can't wedge on xv).
  * V-wave / W2 choreography from v1 (pso1+g2 8-bank wave kt<16,
    bc oc 8-15 full range, g4 merge, W2 with ic order 8..15 first).
  * Rejected (measured or analyzed): AllGather-sharded kT (~100us
    for 0.5MB), fp8 anywhere on the main datapath (3.6% rms rel err
    vs the 2e-2 gate), on-chip PE transpose of x to kill the second
    x load (adds ~50us of LD-bound PE), per-queue DMA streams beyond
    sync (head-of-line or bus contention), V stationary/moving flip
    (breaks the [d, q] layout W2 consumes).
"""

from contextlib import ExitStack

import numpy as np
import ml_dtypes

import concourse.bass as bass
import concourse.bacc as bacc
import concourse.mybir as mybir
import concourse.tile as tile
from concourse.bass_utils import run_bass_kernel_spmd
from concourse.tile_rust import add_dep_helper

N_CTX = 4096
D_MODEL = 2048
D_HEAD = 128
NCORES = 8
QPC = N_CTX // NCORES          # 512 queries per core
NKT = N_CTX // 128             # 32 key tiles
NDM = D_MODEL // 128           # 16 d_model chunks
KG = 128                       # kT projection key-group width
NKG = N_CTX // KG
MASK_NEG = -1.0e30

F16 = mybir.dt.float16
BF16 = mybir.dt.bfloat16
F32 = mybir.dt.float32


def _widths():
    # valid query-column width per key tile (tail slice [512-w : 512] of qT)
    return [QPC - 16 * kt for kt in range(NKT)]


def build_program():
    nc = bacc.Bacc(trn_type="TRN2", target_bir_lowering=False, debug=False)

    # ---- DRAM parameters (identical shapes on all cores; data differs) ----
    # xqr[r, 512*ic + m] = x[8m+c, 128*ic + r]   (own-query columns, packed)
    xqr = nc.declare_dram_parameter("xqr", [128, NDM * QPC], F16, isOutput=False)
    # xtp[ch][r, 2048*j + KG*ic + n] = x[KG*(4*ch+j) + n, 128*ic + r]
    # (chunk-major: 4 key-groups per contiguous 2MB chunk)
    xtp = nc.declare_dram_parameter(
        "xtp", [NKG // 4, 128, 4 * NDM * KG], F16, isOutput=False
    )
    # xv = x (natural layout), bf16
    xv = nc.declare_dram_parameter("xv", [N_CTX, D_MODEL], BF16, isOutput=False)
    # wqr[r, 128*ic + h] = Wq[h, 128*ic + r]; same for wkr
    wqr = nc.declare_dram_parameter("wqr", [128, D_MODEL], F16, isOutput=False)
    wkr = nc.declare_dram_parameter("wkr", [128, D_MODEL], F16, isOutput=False)
    # w2r[oc][r, 128*ic + o] = W2[128*oc + o, 128*ic + r]
    w2r = nc.declare_dram_parameter("w2r", [NDM, 128, D_MODEL], F16, isOutput=False)
    maskb = nc.declare_dram_parameter("maskb", [128, 16], F32, isOutput=False)
    outT = nc.declare_dram_parameter("outT", [D_MODEL, QPC], F16, isOutput=True)

    W = _widths()
    HK = NKT // 2

    with tile.TileContext(nc) as tc:
        with (
            tc.tile_pool(name="static", bufs=1) as st,
            tc.tile_pool(name="xvp1", bufs=24) as xvp1,
            tc.tile_pool(name="xvp2", bufs=8) as xvp2,
            tc.tile_pool(name="ktpool", bufs=8) as ktp,
            tc.tile_pool(name="atpool", bufs=1) as atp,
        ):
            qT_sb = st.tile([128, QPC], F16, tag="qT")
            ones_sb = st.tile([128, 1], BF16, tag="ones")
            mask_sb = st.tile([128, 16], F32, tag="mask")
            recip_sb = st.tile([128, QPC], F32, tag="recip")
            nc.vector.memset(ones_sb[:], 1.0)

            # ---- critical small loads first (sync queue order == byte order)
            nc.sync.dma_start(out=mask_sb[:], in_=maskb[:])

            es1 = ExitStack()  # xts staging + wk/wq: freed before p34
            xts = es1.enter_context(tc.tile_pool(name="xts", bufs=2))
            wkp = es1.enter_context(tc.tile_pool(name="wkp", bufs=1))
            wk_sb = wkp.tile([128, D_MODEL], F16, tag="wk")
            wq_sb = wkp.tile([128, D_MODEL], F16, tag="wq")
            es_xq = ExitStack()  # one-shot xq staging, closed right after qT
            xqp = es_xq.enter_context(tc.tile_pool(name="xqp", bufs=1))
            xq_sb = xqp.tile([128, NDM * QPC], F16, tag="xq")

            # byte order: xq h1 -> wkr -> wqr -> xtp kg 0..3 (per-kg
            # sub-DMAs of chunk 0) -> xq h2 -> xtp chunks 1..7 (2MB each;
            # 5.0us transfer + 0.6us issue = ~1.4us/tile delivery, faster
            # than the ~1.67us/tile PE consumption, so the stream runs
            # ahead and the fused phase never stalls) -> xv singles.
            # xts is one big 8-slot tile; chunk c lands in slots
            # (4c)%8..(4c)%8+3, so every chunk is slot-contiguous.
            # head bytes finely interleaved: qT and kT inputs land
            # progressively from ~10us instead of one 12us xq stall
            QQ = NDM * QPC // 4
            HQ = NDM * QPC // 2
            nc.sync.dma_start(out=wk_sb[:], in_=wkr[:])
            nc.sync.dma_start(out=wq_sb[:], in_=wqr[:])
            nc.sync.dma_start(out=xq_sb[:, 0:QQ], in_=xqr[:, 0:QQ])

            # chunk tiles: 4 key groups each, double-buffered pool so Tile's
            # slot versioning orders chunk c's DMA after chunk c-2's reads
            SW = NDM * KG  # slot width (columns per key group)
            ch_t = []
            ch_dma = []

            def xts_slot(kg):
                return ch_t[kg // 4][:, (kg % 4) * SW : (kg % 4 + 1) * SW]

            t0 = xts.tile([128, 4 * SW], F16, tag="xts", name="xts_ch0")
            ch_t.append(t0)
            d = nc.sync.dma_start(out=t0[:, 0:SW], in_=xtp[0][:, 0:SW])
            nc.sync.dma_start(out=xq_sb[:, QQ:HQ], in_=xqr[:, QQ:HQ])
            d = nc.sync.dma_start(out=t0[:, SW : 2 * SW], in_=xtp[0][:, SW : 2 * SW])
            nc.sync.dma_start(out=xq_sb[:, HQ:], in_=xqr[:, HQ:])
            for kg in range(2, 4):
                d = nc.sync.dma_start(
                    out=t0[:, kg * SW : (kg + 1) * SW],
                    in_=xtp[0][:, kg * SW : (kg + 1) * SW],
                )
            ch_dma.append(d)
            for ch in range(1, NKG // 4):
                t = xts.tile([128, 4 * SW], F16, tag="xts", name=f"xts_ch{ch}")
                ch_t.append(t)
                nc.sync.dma_start(
                    out=t[:, 0 : 2 * SW], in_=xtp[ch][:, 0 : 2 * SW]
                )
                ch_dma.append(
                    nc.sync.dma_start(
                        out=t[:, 2 * SW : 4 * SW], in_=xtp[ch][:, 2 * SW : 4 * SW]
                    )
                )

            xv_t = []
            for kt in range(NKT):
                pool = xvp1 if kt < 24 else xvp2
                t = pool.tile([128, D_MODEL], BF16, tag="xv", name=f"xv{kt}")
                nc.sync.dma_start(out=t[:], in_=xv[128 * kt : 128 * (kt + 1), :])
                xv_t.append(t)

            # ---- fused pipeline: kT proj / qT proj / scores / exp / denom.
            # Emission order == PE execution order: kT[0] first (its inputs
            # land earliest), qT halves woven into kg 0/1, scores trailing
            # the kT projection by two tiles. ----
            at_t = []
            with tc.tile_pool(name="psk", bufs=3, space="PSUM") as pskp:
                es_psq = ExitStack()  # psq bank, freed for pss/psd after qT
                psqp = es_psq.enter_context(
                    tc.tile_pool(name="psq", bufs=1, space="PSUM")
                )
                psq = psqp.tile([128, QPC], F32, tag="psq")
                es_sd = ExitStack()  # pss + psd, opened once psq closes
                pssp = None
                psd = None

                def emit_scores(kt):
                    w = W[kt]
                    ps = pssp.tile([128, 512], F32, tag="pss", name=f"pss{kt}")
                    nc.tensor.matmul(
                        ps[:, :w],
                        kt_tiles[kt][:],
                        qT_sb[:, QPC - w : QPC],
                        start=True, stop=True,
                    )
                    nc.vector.tensor_add(ps[:, :16], ps[:, :16], mask_sb[:])
                    at = atp.tile([128, w], BF16, tag=f"at{kt}")
                    nc.scalar.activation(
                        at[:], ps[:, :w], mybir.ActivationFunctionType.Exp
                    )
                    at_t.append(at)
                    nc.tensor.matmul(
                        psd[0:1, QPC - w : QPC],
                        ones_sb[:],
                        at[:],
                        start=(kt == 0), stop=(kt == NKT - 1),
                    )

                kt_tiles = []
                for kg in range(NKG):
                    psk = pskp.tile([128, KG], F32, tag="psk", name=f"psk{kg}")
                    slot = xts_slot(kg)
                    for ic in range(NDM):
                        nc.tensor.matmul(
                            psk[:],
                            wk_sb[:, 128 * ic : 128 * (ic + 1)],
                            slot[:, KG * ic : KG * (ic + 1)],
                            start=(ic == 0), stop=(ic == NDM - 1),
                        )
                    ktile = ktp.tile([128, KG], F16, tag="kt", name=f"kt{kg}")
                    nc.vector.tensor_copy(ktile[:], psk[:])
                    kt_tiles.append(ktile)

                    # weave the qT projection into the first two kg steps
                    if kg == 0:
                        for ic in range(NDM // 2):
                            nc.tensor.matmul(
                                psq[:],
                                wq_sb[:, 128 * ic : 128 * (ic + 1)],
                                xq_sb[:, QPC * ic : QPC * (ic + 1)],
                                start=(ic == 0), stop=False,
                            )
                    elif kg == 1:
                        for ic in range(NDM // 2, NDM):
                            nc.tensor.matmul(
                                psq[:],
                                wq_sb[:, 128 * ic : 128 * (ic + 1)],
                                xq_sb[:, QPC * ic : QPC * (ic + 1)],
                                start=False, stop=(ic == NDM - 1),
                            )
                        nc.vector.tensor_copy(qT_sb[:], psq[:])
                        es_xq.close()  # free the xq staging (SBUF)
                        es_psq.close()  # free the psq bank ...
                        pssp = es_sd.enter_context(
                            tc.tile_pool(name="pss", bufs=2, space="PSUM")
                        )
                        psdp = es_sd.enter_context(
                            tc.tile_pool(name="psd", bufs=1, space="PSUM")
                        )
                        psd = psdp.tile([1, QPC], F32, tag="psd")
                    if kg >= 2:
                        emit_scores(kg - 2)
                emit_scores(NKT - 2)
                emit_scores(NKT - 1)

                # softmax denominators -> reciprocal (reads psd before the
                # pool closes; broadcast follows outside)
                nc.vector.reciprocal(recip_sb[0:1, :], psd[0:1, :])
                es_sd.close()

            es1.close()  # free wk/wq + xts SBUF for the aoT / W2 pools
            nc.gpsimd.partition_broadcast(recip_sb[:], recip_sb[0:1, :])

            with tc.tile_pool(name="p34", bufs=1) as p34:
                ao_t = {}

                # ---- oc 0-7, first key half (kt 0-15): pso1 + g2 run as
                # one 8-bank wave on the PSUM freed by the fused pools ----
                es2 = ExitStack()  # psv1: freed between the pso1 and g2 evictions
                psv1 = es2.enter_context(
                    tc.tile_pool(name="psv1", bufs=4, space="PSUM", side="right")
                )
                pso1 = [
                    psv1.tile([128, QPC], F32, tag="pso1", name=f"pso1_{j}")
                    for j in range(4)
                ]
                with tc.tile_pool(name="g2", bufs=4, space="PSUM") as g2p:
                    g2 = {
                        oc: g2p.tile([128, QPC], F32, tag="g2", name=f"g2_{oc}")
                        for oc in range(4, 8)
                    }
                    for kt in range(HK):
                        w = W[kt]
                        for j in range(4):
                            nc.tensor.matmul(
                                pso1[j][:, QPC - w : QPC],
                                xv_t[kt][:, 128 * j : 128 * (j + 1)],
                                at_t[kt][:],
                                start=(kt == 0), stop=(kt == HK - 1),
                            )
                        for oc in range(4, 8):
                            nc.tensor.matmul(
                                g2[oc][:, QPC - w : QPC],
                                xv_t[kt][:, 128 * oc : 128 * (oc + 1)],
                                at_t[kt][:],
                                start=(kt == 0), stop=(kt == HK - 1),
                            )
                    # early normalized evictions for oc 0-7 (partial over the
                    # first key half; exact for queries m<256 by causality)
                    for j in range(4):
                        t = p34.tile([128, QPC], F16, tag=f"ao{j}")
                        nc.vector.tensor_mul(t[:], pso1[j][:], recip_sb[:])
                        ao_t[j] = t
                    es2.close()  # release the fused V banks
                    for oc in range(4, 8):
                        t = p34.tile([128, QPC], F16, tag=f"ao{oc}")
                        nc.vector.tensor_mul(t[:], g2[oc][:], recip_sb[:])
                        ao_t[oc] = t

                # ---- oc 8-15, FULL key range: 8 concurrent streams on the
                # freed banks ----
                with (
                    tc.tile_pool(name="bcL", bufs=4, space="PSUM") as bcL,
                    tc.tile_pool(name="bcR", bufs=4, space="PSUM", side="right") as bcR,
                ):
                    bc = {}
                    for i, oc in enumerate(range(8, NDM)):
                        pool = bcL if i < 4 else bcR
                        bc[oc] = pool.tile(
                            [128, QPC], F32, tag="bc", name=f"bc_{oc}"
                        )
                    for kt in range(NKT):
                        w = W[kt]
                        for oc in range(8, NDM):
                            nc.tensor.matmul(
                                bc[oc][:, QPC - w : QPC],
                                xv_t[kt][:, 128 * oc : 128 * (oc + 1)],
                                at_t[kt][:],
                                start=(kt == 0), stop=(kt == NKT - 1),
                            )
                    for oc in range(8, NDM):
                        t = p34.tile([128, QPC], F16, tag=f"ao{oc}")
                        nc.vector.tensor_mul(t[:], bc[oc][:], recip_sb[:])
                        ao_t[oc] = t

                # ---- oc 0-7, second key half (kt 16-31, queries [256:512]
                # only): dense from resident xv, merged into the early aos ----
                with (
                    tc.tile_pool(name="g4", bufs=4, space="PSUM") as g4p,
                    tc.tile_pool(name="tmr", bufs=4) as tmr,
                ):
                    for ocs in (range(0, 4), range(4, 8)):
                        g4 = {
                            oc: g4p.tile(
                                [128, QPC // 2], F32, tag="g4", name=f"g4_{oc}"
                            )
                            for oc in ocs
                        }
                        for kt in range(HK, NKT):
                            w = W[kt]
                            for oc in ocs:
                                nc.tensor.matmul(
                                    g4[oc][:, QPC // 2 - w : QPC // 2],
                                    xv_t[kt][:, 128 * oc : 128 * (oc + 1)],
                                    at_t[kt][:],
                                    start=(kt == HK), stop=(kt == NKT - 1),
                                )
                        for oc in ocs:
                            tm = tmr.tile([128, QPC // 2], F16, tag="tm")
                            nc.vector.tensor_mul(
                                tm[:], g4[oc][:], recip_sb[:, QPC // 2 :]
                            )
                            nc.vector.tensor_add(
                                ao_t[oc][:, QPC // 2 :],
                                ao_t[oc][:, QPC // 2 :],
                                tm[:],
                            )

                    # ---- W2: outT = W2T.T @ attn_outT.  ic order 8..15 first
                    # (those aos finish earliest), 0..7 after the merges ----
                    with (
                        tc.tile_pool(name="w2s", bufs=4) as w2s,
                        tc.tile_pool(name="outs", bufs=4) as outs,
                        tc.tile_pool(name="ps4", bufs=4, space="PSUM", side="right") as ps4,
                    ):
                        ic_order = list(range(8, NDM)) + list(range(0, 8))
                        for oc in range(NDM):
                            tw = w2s.tile([128, D_MODEL], F16, tag="w2")
                            nc.sync.dma_start(out=tw[:], in_=w2r[oc])
                            ps = ps4.tile([128, QPC], F32, tag="ps4")
                            for i, ic in enumerate(ic_order):
                                nc.tensor.matmul(
                                    ps[:],
                                    tw[:, 128 * ic : 128 * (ic + 1)],
                                    ao_t[ic][:],
                                    start=(i == 0), stop=(i == NDM - 1),
                                )
                            t = outs.tile([128, QPC], F16, tag="out")
                            nc.vector.tensor_copy(t[:], ps[:])
                            nc.scalar.dma_start(
                                out=outT[128 * oc : 128 * (oc + 1), :], in_=t[:]
                            )

    nc.compile()
    return nc


def prepare_inputs(x, Wk, Wq, W2):
    """Host-side sharding/layout prep. Returns in_maps for the 8 cores."""
    x = np.asarray(x, dtype=np.float32)
    Wk = np.asarray(Wk, dtype=np.float32)
    Wq = np.asarray(Wq, dtype=np.float32)
    W2 = np.asarray(W2, dtype=np.float32)

    xT16 = np.ascontiguousarray(x.T).astype(np.float16)          # [D, N]
    # per-kg tiles: t[kg, r, KG*ic + n] = xT[128*ic + r, KG*kg + n],
    # then packed chunk-major: xtp[ch, r, SW*j + col] = t[4*ch + j, r, col]
    t = xT16.reshape(NDM, 128, NKG, KG).transpose(2, 1, 0, 3).reshape(NKG, 128, NDM * KG)
    xtp = np.ascontiguousarray(
        t.reshape(NKG // 4, 4, 128, NDM * KG).transpose(0, 2, 1, 3)
        .reshape(NKG // 4, 128, 4 * NDM * KG)
    )
    xv16 = x.astype(ml_dtypes.bfloat16)                          # [N, D]

    def pack_chunks(aT, width):
        # aT [D_MODEL, width] -> [128, NDM*width]: out[r, width*ic + c] = aT[128ic+r, c]
        return np.ascontiguousarray(
            aT.reshape(NDM, 128, width).transpose(1, 0, 2).reshape(128, NDM * width)
        )

    wqr = pack_chunks(np.ascontiguousarray(Wq.T).astype(np.float16), D_HEAD)
    wkr = pack_chunks(np.ascontiguousarray(Wk.T).astype(np.float16), D_HEAD)
    # w2r[oc, r, 128*ic + o] = W2T[128ic+r, 128oc+o]
    w2T = np.ascontiguousarray(W2.T).astype(np.float16)
    w2r = np.ascontiguousarray(
        w2T.reshape(NDM, 128, NDM, 128).transpose(2, 1, 0, 3).reshape(NDM, 128, D_MODEL)
    )

    in_maps = []
    for c in range(NCORES):
        xqT = np.ascontiguousarray(x[c::NCORES].T).astype(np.float16)  # [D, QPC]
        xqr_c = pack_chunks(xqT, QPC)
        mask = np.zeros((128, 16), dtype=np.float32)
        j = np.arange(128)[:, None]
        t = np.arange(16)[None, :]
        mask[j > 8 * t + c] = MASK_NEG
        in_maps.append(
            {
                "xqr": xqr_c,
                "xtp": xtp,
                "xv": xv16,
                "wqr": wqr,
                "wkr": wkr,
                "w2r": w2r,
                "maskb": mask,
            }
        )
    return in_maps


def assemble_output(results):
    res = np.stack([np.asarray(results[c]["outT"]).astype(np.float32) for c in range(NCORES)])
    # [c, d, m] -> out[8m+c, d]
    return np.ascontiguousarray(res.transpose(2, 0, 1).reshape(N_CTX, D_MODEL))


_CACHED = {}


def kernel(x, Wk, Wq, W2, _trace=False):
    if "nc" not in _CACHED:
        _CACHED["nc"] = build_program()
    nc = _CACHED["nc"]
    in_maps = prepare_inputs(x, Wk, Wq, W2)
    res = run_bass_kernel_spmd(nc, in_maps, core_ids=list(range(NCORES)), trace=_trace)
    out = assemble_output(res.results)
    if _trace:
        return out, res
    return out
